# revision 20
# baseline (speedup 1.0000x reference)
"""Trainium2 Bass kernel for nn_AcrBertModel (ragged span mean-pool + MLP head).

out[b] = sigmoid(W2^T relu(W1^T concat(cls_b, mean_b) + b1) + b2)
  cls_b  = features[b, 0, :]
  mean_b = mean over s in [start_b, end_b) of features[b, s, :]

Strategy (8 NeuronCores, data-parallel over batch):
  - Only ~4160 of 65536 token rows per core are needed (spans are <= 64
    tokens inside the first 264 positions).  Each core dma_gathers exactly
    its span rows (ragged-packed: gathered row j -> partition j%128, free
    block j//128) from its HBM-resident feature slice -- ~13 MB instead of
    192 MB per core.
  - Span sums via PE matmuls with a one-hot "owner" mask as the stationary
    operand: mask[k, m] = 1 iff gathered row k belongs to example slot m.
    Masks are generated on device (iota vs per-partition owner id,
    tensor_scalar is_equal) -- no mask DMA.  All tiles accumulate into one
    PSUM [128ex, 768]; the 1/len scaling happens during the PSUM->SBUF
    copy.
  - CLS rows via one strided DMA.  PE transposes give X^T chunks, 12
    accumulating matmuls apply W1, ScalarE relu+bias, one matmul applies
    W2, ScalarE sigmoid.
  - Examples are greedily balanced across the 16 (core, gather-half)
    buckets by span length so every core does the same amount of DMA/PE
    work.  Host undoes the permutation when assembling the output.
"""

import numpy as np
from contextlib import ExitStack

B, S, H = 1024, 512, 768
D1 = 128
NCORES = 8
BPC = B // NCORES  # 128 examples per core
HALF = 64          # examples per gather half (int16 row-index limit: 64*512-1 = 32767)
NCHUNK = (2 * H) // 128  # 12 chunks of the concat feature dim
SPLITS = 8         # sub-gathers per half (DMA/PE overlap granularity)

_PROGRAM_CACHE: dict = {}
LAST_RESULTS = None  # BassKernelResults of the most recent run (for test harness)


def _plan_buckets(lens: np.ndarray):
    """Greedy-balance example indices into 16 buckets (core-major, then half)
    of HALF examples each, minimizing the max bucket span-length sum."""
    nb = NCORES * 2
    order = np.argsort(-lens, kind="stable")
    bsum = np.zeros(nb, dtype=np.int64)
    bcnt = np.zeros(nb, dtype=np.int64)
    buckets = [[] for _ in range(nb)]
    for e in order:
        best, best_s = -1, None
        for i in range(nb):
            if bcnt[i] < HALF and (best_s is None or bsum[i] < best_s):
                best, best_s = i, bsum[i]
        buckets[best].append(int(e))
        bsum[best] += int(lens[e])
        bcnt[best] += 1
    T = int(np.ceil(bsum.max() / 128.0))
    return buckets, T


def _wrap_idx(arr: np.ndarray) -> np.ndarray:
    """int16 index list -> [128, n/16] wrapped layout (j -> partition j%16,
    free j//16), replicated across the 8 groups of 16 partitions."""
    assert arr.size % 16 == 0
    w = arr.reshape(-1, 16).T.astype(np.int16)  # [16, n/16]
    return np.tile(w, (8, 1))  # [128, n/16]


def _splits(T: int):
    """Tile-index boundaries of the SPLITS sub-gathers within one half."""
    return [T * g // SPLITS for g in range(SPLITS + 1)]


def _build_program(T: int, use_f32r: bool):
    import concourse.tile as tile
    from concourse import bacc, mybir
    from concourse.bass import MemorySpace

    f32 = mybir.dt.float32
    nc = bacc.Bacc("TRN2")

    # aux column layout (all small fp32 tensors packed into one DMA):
    #   [0:1536)        w1t   (12 chunks of W1^T, [128, 12, 128])
    #   [1536:1664)     iota  (iota[p, m] = m)
    #   [1664:1792)     identity
    #   [1792:1792+2T)  ownr  (owner id per gathered slot, -1 = padding)
    #   [+0] b1  [+1] w2  [+2] invl  [+3] b2 (row 0 only)
    naux = 1792 + 2 * T + 4
    C_IOT, C_ID, C_OWN = 1536, 1664, 1792
    C_B1, C_W2, C_INV, C_B2 = (
        1792 + 2 * T,
        1793 + 2 * T,
        1794 + 2 * T,
        1795 + 2 * T,
    )

    feat = nc.dram_tensor("feat", [BPC, S, H], f32, kind="ExternalInput")
    aux = nc.dram_tensor("aux", [128, naux], f32, kind="ExternalInput")
    idx = nc.dram_tensor("idx", [128, 2 * T * 8], mybir.dt.int16, kind="ExternalInput")
    outd = nc.dram_tensor("out", [1, BPC], f32, kind="ExternalOutput")

    bounds = _splits(T)

    with tile.TileContext(nc) as tc, ExitStack() as ctx:
        pool = ctx.enter_context(tc.tile_pool(name="sb", bufs=1))
        psum = ctx.enter_context(tc.tile_pool(name="ps", bufs=1, space=MemorySpace.PSUM))
        psum_t = ctx.enter_context(
            tc.tile_pool(name="pst", bufs=2, space=MemorySpace.PSUM)
        )

        # float32r tiles let the span matmuls run at full PE rate; the bits
        # are plain fp32 -- only the PE multiply mode differs
        fmm = mybir.dt.float32r if use_f32r else f32

        # one packed tile per sub-gather so span matmuls start as soon as
        # their sub-gather lands
        packed = {}
        for h in range(2):
            for g in range(SPLITS):
                nt = bounds[g + 1] - bounds[g]
                packed[(h, g)] = pool.tile(
                    [128, nt, H], fmm, name=f"packed{h}{g}", tag=f"packed{h}{g}"
                )

        aux_sb = pool.tile([128, naux], f32)
        idx_sb = pool.tile([128, 2 * T * 8], mybir.dt.int16)
        mask_sb = pool.tile([128, 2 * T, 128], fmm)
        cls_sb = pool.tile([128, H], f32)
        mean_sb = pool.tile([128, H], f32)
        xt_sb = pool.tile([128, NCHUNK, 128], f32)
        h1_sb = pool.tile([128, 128], f32)
        sig_warm = pool.tile([1, 1], f32)
        res_sb = pool.tile([1, BPC], f32)

        nc.sync.dma_start(idx_sb[:], idx[:])
        nc.sync.dma_start(aux_sb[:], aux[:])
        nc.scalar.dma_start(cls_sb[:], feat[:, 0, :])

        iot_sb = aux_sb[:, C_IOT : C_IOT + 128]
        id_sb = aux_sb[:, C_ID : C_ID + 128]

        # preload the sigmoid activation table while DMA streams
        nc.scalar.activation(
            sig_warm[0:1, :],
            aux_sb[0:1, C_B2 : C_B2 + 1],
            mybir.ActivationFunctionType.Sigmoid,
        )

        # on-device one-hot masks: mask[k, t, m] = (iota[k, m] == ownr[k, t])
        for t in range(2 * T):
            nc.vector.tensor_scalar(
                mask_sb[:, t, :],
                iot_sb,
                aux_sb[:, C_OWN + t : C_OWN + t + 1],
                None,
                mybir.AluOpType.is_equal,
            )

        # CLS half of the MLP depends only on the cheap strided DMA -- do it
        # up front so only the mean half remains after the last gather.
        ps_h1 = psum.tile([128, 128], f32)
        for c in range(6):
            pt = psum_t.tile([128, 128], f32, name=f"ptc{c}", tag="pt")
            nc.tensor.transpose(pt[:, :], cls_sb[:, c * 128 : (c + 1) * 128], id_sb)
            nc.vector.tensor_copy(xt_sb[:, c, :], pt[:, :])
            nc.tensor.matmul(
                ps_h1[:, :],
                aux_sb[:, c * 128 : (c + 1) * 128],
                xt_sb[:, c, :],
                start=(c == 0),
                stop=False,
            )

        # ragged gathers
        for h in range(2):
            src = feat[HALF * h : HALF * (h + 1), :, :].rearrange("e s d -> (e s) d")
            if use_f32r:
                src = src.bitcast(mybir.dt.float32r)
            for g in range(SPLITS):
                a, b = bounds[g], bounds[g + 1]
                if a == b:
                    continue
                n = (b - a) * 128
                ca = (h * T + a) * 8
                cb = (h * T + b) * 8
                nc.gpsimd.dma_gather(
                    packed[(h, g)][:, :, :],
                    src,
                    idx_sb[:, ca:cb],
                    n,
                    n,
                    H,
                )

        # Span sums: accumulate mask^T @ packed into PSUM [128ex, 768]
        # (two banks: columns 0:512 and 512:768).
        ps_a = psum.tile([128, 512], f32)
        ps_b = psum.tile([128, 256], f32)
        nt_total = 2 * T

        ti = 0
        for h in range(2):
            for g in range(SPLITS):
                ptile = packed[(h, g)]
                for tl in range(bounds[g + 1] - bounds[g]):
                    t = h * T + bounds[g] + tl
                    nc.tensor.matmul(
                        ps_a[:, :],
                        mask_sb[:, t, :],
                        ptile[:, tl, 0:512],
                        start=(ti == 0),
                        stop=(ti == nt_total - 1),
                    )
                    nc.tensor.matmul(
                        ps_b[:, :],
                        mask_sb[:, t, :],
                        ptile[:, tl, 512:768],
                        start=(ti == 0),
                        stop=(ti == nt_total - 1),
                    )
                    ti += 1

        # means = span sums * (1/len), scaled during PSUM->SBUF copy
        nc.vector.tensor_scalar(
            mean_sb[:, 0:512], ps_a[:, :], aux_sb[:, C_INV : C_INV + 1], None,
            mybir.AluOpType.mult,
        )
        nc.vector.tensor_scalar(
            mean_sb[:, 512:768], ps_b[:, :], aux_sb[:, C_INV : C_INV + 1], None,
            mybir.AluOpType.mult,
        )

        # mean half of X^T chunks + remaining MLP1 accumulation
        for c in range(6, NCHUNK):
            lo = (c - 6) * 128
            pt = psum_t.tile([128, 128], f32, name=f"ptm{c}", tag="pt")
            nc.tensor.transpose(pt[:, :], mean_sb[:, lo : lo + 128], id_sb)
            nc.vector.tensor_copy(xt_sb[:, c, :], pt[:, :])
            nc.tensor.matmul(
                ps_h1[:, :],
                aux_sb[:, c * 128 : (c + 1) * 128],
                xt_sb[:, c, :],
                start=False,
                stop=(c == NCHUNK - 1),
            )
        # relu(h1 + b1) on DVE (per-partition bias add, then max with 0) --
        # avoids a ScalarE activation-table load in the tail
        nc.vector.tensor_scalar(
            h1_sb[:, :],
            ps_h1[:, :],
            aux_sb[:, C_B1 : C_B1 + 1],
            0.0,
            mybir.AluOpType.add,
            mybir.AluOpType.max,
        )

        # MLP layer 2 + sigmoid.
        ps_out = psum.tile([1, BPC], f32)
        nc.tensor.matmul(
            ps_out[0:1, :],
            aux_sb[:, C_W2 : C_W2 + 1],
            h1_sb[:, :],
            start=True,
            stop=True,
        )
        nc.scalar.activation(
            res_sb[0:1, :],
            ps_out[0:1, :],
            mybir.ActivationFunctionType.Sigmoid,
            bias=aux_sb[0:1, C_B2 : C_B2 + 1],
        )
        nc.sync.dma_start(outd[:], res_sb[0:1, :])

    nc.compile()
    return nc


def _prepare(features, start, end):
    lens = (end - start).astype(np.int64)
    buckets, T = _plan_buckets(lens)

    perm = np.concatenate([np.asarray(b, dtype=np.int64) for b in buckets])
    feat_g = features[perm]  # [B, S, H] permuted so core c owns rows 128c:128c+128

    inv_scale = (1.0 / lens.astype(np.float64)).astype(np.float32)

    in_maps = []
    for c in range(NCORES):
        idx_cols = []
        ownr = np.full((2 * T, 128), -1.0, dtype=np.float32)  # [tile, k]
        for h in range(2):
            bk = buckets[2 * c + h]
            rows = []
            owners = []
            for j, e in enumerate(bk):
                s0, e0 = int(start[e]), int(end[e])
                assert 0 < s0 < e0 <= S
                rows.append(j * S + np.arange(s0, e0, dtype=np.int64))
                owners.append(np.full(e0 - s0, h * HALF + j, dtype=np.int64))
            rows = np.concatenate(rows)
            owners = np.concatenate(owners)
            n = rows.size
            assert n <= T * 128 and rows.max() <= 32767
            rows_p = np.zeros(T * 128, dtype=np.int16)
            rows_p[:n] = rows.astype(np.int16)
            idx_cols.append(_wrap_idx(rows_p))
            ow = np.full(T * 128, -1.0, dtype=np.float32)
            ow[:n] = owners.astype(np.float32)
            ownr[h * T : (h + 1) * T] = ow.reshape(T, 128)
        in_maps.append(
            {
                "feat": feat_g[c * BPC : (c + 1) * BPC],
                "_ownr": np.ascontiguousarray(ownr.T),  # [128 k, 2T]
                "_invl": inv_scale[perm[c * BPC : (c + 1) * BPC]].reshape(128, 1),
                "idx": np.concatenate(idx_cols, axis=1),
            }
        )
    return in_maps, perm, T


def build_in_maps(features, start, end, W1, b1, W2, b2):
    """Full host prep: bucket/balance, gather indices, packed aux tensors.
    Returns (in_maps, perm, T)."""
    in_maps, perm, T = _prepare(features, start, end)

    # pack all small fp32 tensors into one "aux" input per core
    # (column layout must match _build_program)
    naux = 1792 + 2 * T + 4
    base = np.zeros((128, naux), dtype=np.float32)
    base[:, 0:1536] = W1.reshape(NCHUNK, 128, D1).transpose(1, 0, 2).reshape(128, 1536)
    base[:, 1536:1664] = np.arange(128, dtype=np.float32)[None, :]
    base[:, 1664:1792] = np.eye(128, dtype=np.float32)
    base[:, 1792 + 2 * T] = b1
    base[:, 1793 + 2 * T] = W2[:, 0]
    base[0, 1795 + 2 * T] = b2[0]
    for m in in_maps:
        a = base.copy()
        a[:, 1792 : 1792 + 2 * T] = m.pop("_ownr")
        a[:, 1794 + 2 * T] = m.pop("_invl")[:, 0]
        m["aux"] = a
    return in_maps, perm, T


def kernel(
    features_extract,
    start_token_idx,
    end_token_idx,
    W1,
    b1,
    W2,
    b2,
    _trace=False,
    _use_f32r=True,
):
    global LAST_RESULTS
    from concourse.bass_utils import run_bass_kernel_spmd

    features = np.ascontiguousarray(np.asarray(features_extract, dtype=np.float32))
    start = np.asarray(start_token_idx).astype(np.int64)
    end = np.asarray(end_token_idx).astype(np.int64)
    W1 = np.asarray(W1, dtype=np.float32)
    b1 = np.asarray(b1, dtype=np.float32)
    W2 = np.asarray(W2, dtype=np.float32)
    b2 = np.asarray(b2, dtype=np.float32)

    in_maps, perm, T = build_in_maps(features, start, end, W1, b1, W2, b2)

    key = (T, bool(_use_f32r))
    if key not in _PROGRAM_CACHE:
        _PROGRAM_CACHE[key] = _build_program(T, _use_f32r)
    nc = _PROGRAM_CACHE[key]

    res = run_bass_kernel_spmd(nc, in_maps, list(range(NCORES)), trace=_trace)
    LAST_RESULTS = res

    out = np.empty(B, dtype=np.float32)
    for c in range(NCORES):
        out[perm[c * BPC : (c + 1) * BPC]] = res.results[c]["out"][0]
    return out.reshape(B, 1, 1)


# revision 34
# speedup vs baseline: 1.1718x; 1.1718x over previous
"""Trainium2 Bass kernel for nn_AcrBertModel (ragged span mean-pool + MLP head).

out[b] = sigmoid(W2^T relu(W1^T concat(cls_b, mean_b) + b1) + b2)
  cls_b  = features[b, 0, :]
  mean_b = mean over s in [start_b, end_b) of features[b, s, :]

Strategy (8 NeuronCores, data-parallel over batch):
  - Only ~4160 of 65536 token rows per core are needed (spans are <= 64
    tokens inside the first 264 positions).  Each core dma_gathers exactly
    its span rows (ragged-packed: gathered row j -> partition j%128, free
    block j//128) from its HBM-resident feature slice -- ~13 MB instead of
    192 MB per core.
  - Span sums via PE matmuls with a one-hot "owner" mask as the stationary
    operand: mask[k, m] = 1 iff gathered row k belongs to example slot m.
    Masks are generated on device (iota vs per-partition owner id,
    tensor_scalar is_equal) -- no mask DMA.  All tiles accumulate into one
    PSUM [128ex, 768]; the 1/len scaling happens during the PSUM->SBUF
    copy.
  - CLS rows via one strided DMA.  PE transposes give X^T chunks, 12
    accumulating matmuls apply W1, ScalarE relu+bias, one matmul applies
    W2, ScalarE sigmoid.
  - Examples are greedily balanced across the 16 (core, gather-half)
    buckets by span length so every core does the same amount of DMA/PE
    work.  Host undoes the permutation when assembling the output.
"""

import numpy as np
from contextlib import ExitStack

B, S, H = 1024, 512, 768
D1 = 128
NCORES = 8
BPC = B // NCORES  # 128 examples per core
HALF = 64          # examples per gather half (int16 row-index limit: 64*512-1 = 32767)
NCHUNK = (2 * H) // 128  # 12 chunks of the concat feature dim
SPLITS = 8         # sub-gathers per half (DMA/PE overlap granularity)
NQUEUES = 2        # SWDGE queues (gathers alternate; parallel Q7 core pairs)
SCRATCH = 32768    # SWDGE descriptor-ring bytes per partition group

_PROGRAM_CACHE: dict = {}
LAST_RESULTS = None  # BassKernelResults of the most recent run (for test harness)


def _plan_buckets(lens: np.ndarray):
    """Greedy-balance example indices into 16 buckets (core-major, then half)
    of HALF examples each, minimizing the max bucket span-length sum."""
    nb = NCORES * 2
    order = np.argsort(-lens, kind="stable")
    bsum = np.zeros(nb, dtype=np.int64)
    bcnt = np.zeros(nb, dtype=np.int64)
    buckets = [[] for _ in range(nb)]
    for e in order:
        best, best_s = -1, None
        for i in range(nb):
            if bcnt[i] < HALF and (best_s is None or bsum[i] < best_s):
                best, best_s = i, bsum[i]
        buckets[best].append(int(e))
        bsum[best] += int(lens[e])
        bcnt[best] += 1
    T = int(np.ceil(bsum.max() / 128.0))
    return buckets, T


def _wrap_idx(arr: np.ndarray) -> np.ndarray:
    """int16 index list -> [128, n/16] wrapped layout (j -> partition j%16,
    free j//16), replicated across the 8 groups of 16 partitions."""
    assert arr.size % 16 == 0
    w = arr.reshape(-1, 16).T.astype(np.int16)  # [16, n/16]
    return np.tile(w, (8, 1))  # [128, n/16]


def _splits(T: int):
    """Tile-index boundaries of the SPLITS sub-gathers within one half."""
    return [T * g // SPLITS for g in range(SPLITS + 1)]


def _build_program(T: int, use_f32r: bool):
    import concourse.tile as tile
    from concourse import bacc, mybir
    from concourse.bass import MemorySpace

    f32 = mybir.dt.float32
    # 2 SWDGE queues + a larger descriptor ring keep the Q7 descriptor
    # generator from stalling the gather pipeline
    nc = bacc.Bacc(
        "TRN2", num_swdge_queues=NQUEUES, dynamic_dma_scratch_size=SCRATCH
    )

    # aux column layout (all small fp32 tensors packed into one DMA):
    #   [0:1536)        w1t   (12 chunks of W1^T, [128, 12, 128])
    #   [1536:1664)     iota  (iota[p, m] = m)
    #   [1664:1792)     identity
    #   [1792:1792+2T)  ownr  (owner id per gathered slot, -1 = padding)
    #   [+0] b1  [+1] w2  [+2] invl  [+3] b2 (row 0 only)
    naux = 1792 + 2 * T + 4
    C_IOT, C_ID, C_OWN = 1536, 1664, 1792
    C_B1, C_W2, C_INV, C_B2 = (
        1792 + 2 * T,
        1793 + 2 * T,
        1794 + 2 * T,
        1795 + 2 * T,
    )

    feat = nc.dram_tensor("feat", [BPC, S, H], f32, kind="ExternalInput")
    aux = nc.dram_tensor("aux", [128, naux], f32, kind="ExternalInput")
    idx = nc.dram_tensor("idx", [128, 2 * T * 8], mybir.dt.int16, kind="ExternalInput")
    outd = nc.dram_tensor("out", [1, BPC], f32, kind="ExternalOutput")

    bounds = _splits(T)

    with tile.TileContext(nc) as tc, ExitStack() as ctx:
        pool = ctx.enter_context(tc.tile_pool(name="sb", bufs=1))
        psum = ctx.enter_context(tc.tile_pool(name="ps", bufs=1, space=MemorySpace.PSUM))
        psum_t = ctx.enter_context(
            tc.tile_pool(name="pst", bufs=2, space=MemorySpace.PSUM)
        )

        # float32r tiles let the span matmuls run at full PE rate; the bits
        # are plain fp32 -- only the PE multiply mode differs
        fmm = mybir.dt.float32r if use_f32r else f32

        # one packed tile per sub-gather so span matmuls start as soon as
        # their sub-gather lands
        packed = {}
        for h in range(2):
            for g in range(SPLITS):
                nt = bounds[g + 1] - bounds[g]
                packed[(h, g)] = pool.tile(
                    [128, nt, H], fmm, name=f"packed{h}{g}", tag=f"packed{h}{g}"
                )

        aux_sb = pool.tile([128, naux], f32)
        idx_sb = pool.tile([128, 2 * T * 8], mybir.dt.int16)
        mask_sb = pool.tile([128, 2 * T, 128], fmm)
        cls_sb = pool.tile([128, H], f32)
        mean_sb = pool.tile([128, H], f32)
        xt_sb = pool.tile([128, NCHUNK, 128], f32)
        h1_sb = pool.tile([128, 128], f32)
        sig_warm = pool.tile([1, 1], f32)
        res_sb = pool.tile([1, BPC], f32)

        nc.scalar.dma_start(idx_sb[:], idx[:])
        nc.sync.dma_start(aux_sb[:], aux[:])
        nc.sync.dma_start(cls_sb[:], feat[:, 0, :])

        # ragged gathers (start as soon as idx lands; alternate SWDGE queues)
        gi = 0
        for h in range(2):
            src = feat[HALF * h : HALF * (h + 1), :, :].rearrange("e s d -> (e s) d")
            if use_f32r:
                src = src.bitcast(mybir.dt.float32r)
            for g in range(SPLITS):
                a, b = bounds[g], bounds[g + 1]
                if a == b:
                    continue
                n = (b - a) * 128
                ca = (h * T + a) * 8
                cb = (h * T + b) * 8
                nc.gpsimd.dma_gather(
                    packed[(h, g)][:, :, :],
                    src,
                    idx_sb[:, ca:cb],
                    n,
                    n,
                    H,
                    queue_num=gi % NQUEUES,
                )
                gi += 1

        iot_sb = aux_sb[:, C_IOT : C_IOT + 128]
        id_sb = aux_sb[:, C_ID : C_ID + 128]

        # preload the sigmoid activation table while DMA streams
        nc.scalar.activation(
            sig_warm[0:1, :],
            aux_sb[0:1, C_B2 : C_B2 + 1],
            mybir.ActivationFunctionType.Sigmoid,
        )

        # on-device one-hot masks: mask[k, t, m] = (iota[k, m] == ownr[k, t]);
        # owner ids are half-local (0..63), so masks are only 64 wide
        for t in range(2 * T):
            nc.vector.tensor_scalar(
                mask_sb[:, t, :],
                iot_sb,
                aux_sb[:, C_OWN + t : C_OWN + t + 1],
                None,
                mybir.AluOpType.is_equal,
            )

        # CLS half of the MLP depends only on the cheap strided DMA -- do it
        # up front so only the mean half remains after the last gather.
        ps_h1 = psum.tile([128, 128], f32)
        for c in range(6):
            pt = psum_t.tile([128, 128], f32, name=f"ptc{c}", tag="pt")
            nc.tensor.transpose(pt[:, :], cls_sb[:, c * 128 : (c + 1) * 128], id_sb)
            nc.vector.tensor_copy(xt_sb[:, c, :], pt[:, :])
            nc.tensor.matmul(
                ps_h1[:, :],
                aux_sb[:, c * 128 : (c + 1) * 128],
                xt_sb[:, c, :],
                start=(c == 0),
                stop=False,
            )

        # Span sums: accumulate mask^T @ packed into PSUM [128ex, 768]
        # (two banks: columns 0:512 and 512:768).
        ps_a = psum.tile([128, 512], f32)
        ps_b = psum.tile([128, 256], f32)
        nt_total = 2 * T

        ti = 0
        for h in range(2):
            for g in range(SPLITS):
                ptile = packed[(h, g)]
                for tl in range(bounds[g + 1] - bounds[g]):
                    t = h * T + bounds[g] + tl
                    nc.tensor.matmul(
                        ps_a[:, :],
                        mask_sb[:, t, :],
                        ptile[:, tl, 0:512],
                        start=(ti == 0),
                        stop=(ti == nt_total - 1),
                    )
                    nc.tensor.matmul(
                        ps_b[:, :],
                        mask_sb[:, t, :],
                        ptile[:, tl, 512:768],
                        start=(ti == 0),
                        stop=(ti == nt_total - 1),
                    )
                    ti += 1

        # means = span sums * (1/len), scaled during PSUM->SBUF copy
        nc.vector.tensor_scalar(
            mean_sb[:, 0:512], ps_a[:, :], aux_sb[:, C_INV : C_INV + 1], None,
            mybir.AluOpType.mult,
        )
        nc.vector.tensor_scalar(
            mean_sb[:, 512:768], ps_b[:, :], aux_sb[:, C_INV : C_INV + 1], None,
            mybir.AluOpType.mult,
        )

        # mean half of X^T chunks + remaining MLP1 accumulation
        for c in range(6, NCHUNK):
            lo = (c - 6) * 128
            pt = psum_t.tile([128, 128], f32, name=f"ptm{c}", tag="pt")
            nc.tensor.transpose(pt[:, :], mean_sb[:, lo : lo + 128], id_sb)
            nc.vector.tensor_copy(xt_sb[:, c, :], pt[:, :])
            nc.tensor.matmul(
                ps_h1[:, :],
                aux_sb[:, c * 128 : (c + 1) * 128],
                xt_sb[:, c, :],
                start=False,
                stop=(c == NCHUNK - 1),
            )
        # relu(h1 + b1) on DVE (per-partition bias add, then max with 0) --
        # avoids a ScalarE activation-table load in the tail
        nc.vector.tensor_scalar(
            h1_sb[:, :],
            ps_h1[:, :],
            aux_sb[:, C_B1 : C_B1 + 1],
            0.0,
            mybir.AluOpType.add,
            mybir.AluOpType.max,
        )

        # MLP layer 2 + sigmoid.
        ps_out = psum.tile([1, BPC], f32)
        nc.tensor.matmul(
            ps_out[0:1, :],
            aux_sb[:, C_W2 : C_W2 + 1],
            h1_sb[:, :],
            start=True,
            stop=True,
        )
        nc.scalar.activation(
            res_sb[0:1, :],
            ps_out[0:1, :],
            mybir.ActivationFunctionType.Sigmoid,
            bias=aux_sb[0:1, C_B2 : C_B2 + 1],
        )
        nc.sync.dma_start(outd[:], res_sb[0:1, :])

    nc.compile()
    return nc


def _prepare(features, start, end):
    lens = (end - start).astype(np.int64)
    buckets, T = _plan_buckets(lens)

    perm = np.concatenate([np.asarray(b, dtype=np.int64) for b in buckets])
    feat_g = features[perm]  # [B, S, H] permuted so core c owns rows 128c:128c+128

    inv_scale = (1.0 / lens.astype(np.float64)).astype(np.float32)

    in_maps = []
    for c in range(NCORES):
        idx_cols = []
        ownr = np.full((2 * T, 128), -1.0, dtype=np.float32)  # [tile, k]
        for h in range(2):
            bk = buckets[2 * c + h]
            rows = []
            owners = []
            for j, e in enumerate(bk):
                s0, e0 = int(start[e]), int(end[e])
                assert 0 < s0 < e0 <= S
                rows.append(j * S + np.arange(s0, e0, dtype=np.int64))
                owners.append(np.full(e0 - s0, h * HALF + j, dtype=np.int64))
            rows = np.concatenate(rows)
            owners = np.concatenate(owners)
            n = rows.size
            assert n <= T * 128 and rows.max() <= 32767
            rows_p = np.zeros(T * 128, dtype=np.int16)
            rows_p[:n] = rows.astype(np.int16)
            idx_cols.append(_wrap_idx(rows_p))
            ow = np.full(T * 128, -1.0, dtype=np.float32)
            ow[:n] = owners.astype(np.float32)
            ownr[h * T : (h + 1) * T] = ow.reshape(T, 128)
        in_maps.append(
            {
                "feat": feat_g[c * BPC : (c + 1) * BPC],
                "_ownr": np.ascontiguousarray(ownr.T),  # [128 k, 2T]
                "_invl": inv_scale[perm[c * BPC : (c + 1) * BPC]].reshape(128, 1),
                "idx": np.concatenate(idx_cols, axis=1),
            }
        )
    return in_maps, perm, T


def build_in_maps(features, start, end, W1, b1, W2, b2):
    """Full host prep: bucket/balance, gather indices, packed aux tensors.
    Returns (in_maps, perm, T)."""
    in_maps, perm, T = _prepare(features, start, end)

    # pack all small fp32 tensors into one "aux" input per core
    # (column layout must match _build_program)
    naux = 1792 + 2 * T + 4
    base = np.zeros((128, naux), dtype=np.float32)
    base[:, 0:1536] = W1.reshape(NCHUNK, 128, D1).transpose(1, 0, 2).reshape(128, 1536)
    base[:, 1536:1664] = np.arange(128, dtype=np.float32)[None, :]
    base[:, 1664:1792] = np.eye(128, dtype=np.float32)
    base[:, 1792 + 2 * T] = b1
    base[:, 1793 + 2 * T] = W2[:, 0]
    base[0, 1795 + 2 * T] = b2[0]
    for m in in_maps:
        a = base.copy()
        a[:, 1792 : 1792 + 2 * T] = m.pop("_ownr")
        a[:, 1794 + 2 * T] = m.pop("_invl")[:, 0]
        m["aux"] = a
    return in_maps, perm, T


def kernel(
    features_extract,
    start_token_idx,
    end_token_idx,
    W1,
    b1,
    W2,
    b2,
    _trace=False,
    _use_f32r=True,
):
    global LAST_RESULTS
    from concourse.bass_utils import run_bass_kernel_spmd

    features = np.ascontiguousarray(np.asarray(features_extract, dtype=np.float32))
    start = np.asarray(start_token_idx).astype(np.int64)
    end = np.asarray(end_token_idx).astype(np.int64)
    W1 = np.asarray(W1, dtype=np.float32)
    b1 = np.asarray(b1, dtype=np.float32)
    W2 = np.asarray(W2, dtype=np.float32)
    b2 = np.asarray(b2, dtype=np.float32)

    in_maps, perm, T = build_in_maps(features, start, end, W1, b1, W2, b2)

    key = (T, bool(_use_f32r))
    if key not in _PROGRAM_CACHE:
        _PROGRAM_CACHE[key] = _build_program(T, _use_f32r)
    nc = _PROGRAM_CACHE[key]

    res = run_bass_kernel_spmd(nc, in_maps, list(range(NCORES)), trace=_trace)
    LAST_RESULTS = res

    out = np.empty(B, dtype=np.float32)
    for c in range(NCORES):
        out[perm[c * BPC : (c + 1) * BPC]] = res.results[c]["out"][0]
    return out.reshape(B, 1, 1)
